# revision 18
# baseline (speedup 1.0000x reference)
"""Trainium2 Bass kernel for nn_Attention_Correlation_weight_reshape_loss.

loss = [ sum|real - C_real| + sum|fake - C_fake| ] / (B*(PP^2-PP))

Key identity: C_IN == C_OUT == 0.8, so with s[b,i] = +1 if fake_weight[b,i] > 0
else -1 the fake target is rank-1:
    C_fake[b,i,j] = 0.45 + 0.35 * s[b,i] * s[b,j]
C_real = 0.8 everywhere except the diagonal (1.0).

Per-core plan (data-parallel over batch, 8 cores x 128 batches):
  real map, layout [batch=partition, 38416 free], 14 chunks:
      ScalarE:  y = Abs(x - 0.8) with free-dim accum -> per-chunk column
      VectorE:  gather diagonal (stride 197) into a compact [128,196] tile
      final:    Abs-accum of |diag-1| and |diag-0.8| (host applies +/-)
  fake map, per-batch row-pair layout [98 partitions, 2 rows, 196] so HBM
  descriptors are 1568B (rows 2p,2p+1 are adjacent in memory):
      TensorE:  one K=2 bf16 matmul per batch: block-diagonal rhs emits
                sigma[p, h*196+j] = s[2p+h]*s[j] into one PSUM bank
      VectorE:  custom fused DVE op over 4 batches at a time:
                out = |sigma*(-0.35) - 0.45 + x|, accum_out = row sum
Host sums the [128, 48] partial tensor from each core and divides by denom.
"""

from operator import add as _op_add

import numpy as np

import concourse.bacc as bacc
import concourse.bass as bass
import concourse.mybir as mybir
import concourse.tile as tile
from concourse import bass_utils
from concourse import dve_ops as _dops
from concourse.dve_spec import Spec, Src0, Src1, Zero, maxx, lower
from concourse.dve_spec import _has_src1
from concourse import dve_spec as _dspec
from concourse.dve_uop import DveOpSpec
from concourse.dve_table_gen import dve_ver_for

F32 = mybir.dt.float32
BF16 = mybir.dt.bfloat16
AF = mybir.ActivationFunctionType
ALU = mybir.AluOpType

B, PP = 1024, 196
NCORES = 8
BS = B // NCORES            # 128 batches per core
FF = PP * PP                # 38416
RC = 7                      # real chunks
RF = FF // RC               # 5488
HALF = PP // 2              # 98
GRP = 8                     # fake batches per DMA group
NGRP = BS // GRP            # 16
DVB = 4                     # fake batches per custom-DVE op (PSUM banks)
NOPS = BS // DVB            # 32 fake accum columns

# output partials layout: [128, NCOL]
COL_REAL = 0                # RC cols: per-chunk sum|x-0.8| (incl diag)
COL_FAKE = RC               # 32 cols: per-4-batch fake sums (rows 0..97 only)
COL_D1 = RC + NOPS          # sum|diag-1.0|
COL_D8 = COL_D1 + 1        # sum|diag-0.8|
NCOL = COL_D8 + 1          # 48

DENOM = float(B) * (FF - PP)

_NC_CACHE = {}


def _register_abs_op():
    """Custom DVE op: out = |(in0*s0 - s1) + in1|, accum_out = row-sum(out)."""
    name = "ABS_AFFINE_SUM_ANT"
    for op in _dops.OPS:
        if op.name == name:
            return op
    e = (Src0 * _dspec.C0 - _dspec.C1) + Src1
    body = maxx(e, Zero - e)

    def _ref(in0, in1, c0, c1, c2):
        P = in0.shape[0]
        a = np.asarray(in0, dtype=np.float32).reshape(P, -1)
        x = np.asarray(in1, dtype=np.float32).reshape(P, -1)
        b = np.abs((a * c0 - c1) + x).astype(np.float32)
        return b, b.sum(axis=-1, keepdims=True)

    spec = Spec(body=body, accum=_op_add, accum_init=Zero, reference=_ref)
    row = max(_dops._SUB_OPCODE_FOR_NAME.values()) + 1
    assert row < 0x20
    _dops._SUB_OPCODE_FOR_NAME[name] = row
    shas = {}
    for ver in ("v3", "v4"):
        s = DveOpSpec(
            name=name, opcode=row, uops=lower(spec, ver=ver),
            rd1_en=_has_src1(spec),
        )
        shas[ver] = s.sha(ver)
    op = _dops.DveOp(name, spec, subdim=False, uops_sha=shas)
    _dops.OPS.append(op)
    _dops.CUSTOM_DVE_SPECS[name] = spec
    return op


def build_nc():
    abs_op = _register_abs_op()
    nc = bacc.Bacc(
        "TRN2", target_bir_lowering=False, debug=False, enable_asserts=False
    )
    real = nc.dram_tensor("real", [BS, FF], F32, kind="ExternalInput").ap()
    fake = nc.dram_tensor("fake", [BS, PP, PP], F32, kind="ExternalInput").ap()
    fw = nc.dram_tensor("fw", [BS, PP], F32, kind="ExternalInput").ap()
    out = nc.dram_tensor("out", [128, NCOL], F32, kind="ExternalOutput").ap()

    with tile.TileContext(nc) as tc:
        with (
            tc.tile_pool(name="small", bufs=1) as sp,
            tc.tile_pool(name="xr", bufs=2) as xr_pool,
            tc.tile_pool(name="yr", bufs=1) as yr_pool,
            tc.tile_pool(name="xf", bufs=3) as xf_pool,
            tc.tile_pool(name="d", bufs=2) as d_pool,
            tc.tile_pool(name="ps", bufs=2, space="PSUM") as ps_pool,
        ):
            O = sp.tile([128, NCOL], F32)
            nc.gpsimd.memset(O[:], 0.0)

            # bias constants for scalar-engine activations ([P,1] APs)
            b08 = sp.tile([128, 1], F32)
            nc.gpsimd.memset(b08[:], -0.8)
            bm1 = sp.tile([128, 1], F32)
            nc.gpsimd.memset(bm1[:], -1.0)

            # --- s prep: s = +/-1 (bf16) from fw > 0.  All small DMAs go
            # through GPSIMD (SWDGE) so the sync/scalar HWDGE streams start
            # on the big loads immediately.
            fwt = sp.tile([128, PP], F32)
            nc.gpsimd.dma_start(fwt[:], fw[:, :])
            g_t = sp.tile([128, PP], F32)
            nc.vector.tensor_scalar(g_t[:], fwt[:], 0.0, None, ALU.is_gt)
            s_bf = sp.tile([128, PP], BF16)
            nc.vector.tensor_scalar(s_bf[:], g_t[:], 2.0, 1.0, ALU.mult, ALU.subtract)
            # even/odd row weights for the K=2 row-pair matmul, flattened
            # batch-major onto partitions 0 and 1
            e_t = sp.tile([128, HALF], BF16)
            nc.vector.tensor_copy(e_t[:], s_bf[:, 0 : PP - 1 : 2])
            o_t = sp.tile([128, HALF], BF16)
            nc.vector.tensor_copy(o_t[:], s_bf[:, 1:PP:2])
            lhsT2 = sp.tile([2, BS * HALF], BF16)
            nc.gpsimd.dma_start(lhsT2[0:1, :], e_t[:])
            nc.gpsimd.dma_start(lhsT2[1:2, :], o_t[:])

            # two persistent block-diagonal rhs tiles (zeros memset once;
            # per-group DMAs only overwrite the s blocks)
            rhs_tiles = []
            for _ in range(2):
                rt = sp.tile([2, GRP * 2 * PP], BF16, tag=f"rhs{_}")
                nc.gpsimd.memset(rt[:], 0.0)
                rhs_tiles.append(rt[:].rearrange("p (g j) -> p g j", g=GRP))

            diag = sp.tile([128, PP], F32)

            def real_chunk(c):
                xr = xr_pool.tile([128, RF], F32, tag="xr")
                nc.scalar.dma_start(xr[:], real[:, c * RF : (c + 1) * RF])
                yr = yr_pool.tile([128, RF], F32, tag="yr")
                nc.scalar.activation(
                    yr[:], xr[:], AF.Abs, bias=b08[:],
                    accum_out=O[:, COL_REAL + c : COL_REAL + c + 1],
                )
                # diagonal positions f = 197*i inside this chunk
                lo = c * RF
                i0 = -(-lo // 197)
                i1 = -(-(lo + RF) // 197)
                off = 197 * i0 - lo
                cnt = i1 - i0
                nc.vector.tensor_copy(
                    diag[:, i0:i1], xr[:, off : off + 197 * (cnt - 1) + 1 : 197]
                )

            def fake_dma(gr):
                b0 = gr * GRP
                # row-pair fold: xf[p, g, h, j] = fake[b0+g, 2p+h, j]
                xf = xf_pool.tile([HALF, GRP, 2, PP], F32, tag="xf")
                nc.sync.dma_start(
                    xf[:],
                    fake[b0 : b0 + GRP, :, :].rearrange(
                        "g (p h) j -> p g h j", h=2
                    ),
                )
                return xf

            def fake_compute(gr, xf):
                b0 = gr * GRP
                # block-diagonal rhs: per batch block [s_b | 0 ; 0 | s_b]
                r3 = rhs_tiles[gr % 2]
                nc.gpsimd.dma_start(r3[0:1, :, 0:PP], s_bf[b0 : b0 + GRP, :])
                nc.gpsimd.dma_start(
                    r3[1:2, :, PP : 2 * PP], s_bf[b0 : b0 + GRP, :]
                )
                for m in range(GRP // DVB):
                    ps4 = ps_pool.tile([HALF, DVB, 512], F32, tag="ps")
                    for bl in range(DVB):
                        gi = m * DVB + bl
                        b = b0 + gi
                        nc.tensor.matmul(
                            ps4[:, bl, 0 : 2 * PP],
                            lhsT2[0:2, b * HALF : (b + 1) * HALF],
                            r3[0:2, gi, :],
                            start=True, stop=True,
                        )
                    d = d_pool.tile([HALF, DVB, 2 * PP], F32, tag="d")
                    col = COL_FAKE + gr * (GRP // DVB) + m
                    nc.vector._custom_dve(
                        abs_op,
                        out=d[:],
                        in0=ps4[:, :, 0 : 2 * PP],
                        in1=xf[:, m * DVB : (m + 1) * DVB, :, :].rearrange(
                            "p g h j -> p g (h j)"
                        ),
                        s0=-0.35,
                        s1=0.45,
                        accum_out=O[0:HALF, col : col + 1],
                    )

            # interleave: 16 fake groups, RC real chunks spread 1-per-2-groups
            pending = []
            for i in range(NGRP):
                pending.append((i, fake_dma(i)))
                if i % 2 == 0 and i // 2 < RC:
                    real_chunk(i // 2)
                while pending:
                    gr, xf = pending.pop(0)
                    fake_compute(gr, xf)

            # diagonal corrections: real diag target is 1.0 (not 0.8)
            t1 = sp.tile([128, PP], F32)
            nc.scalar.activation(
                t1[:], diag[:], AF.Abs, bias=bm1[:],
                accum_out=O[:, COL_D1 : COL_D1 + 1],
            )
            t2 = sp.tile([128, PP], F32)
            nc.scalar.activation(
                t2[:], diag[:], AF.Abs, bias=b08[:],
                accum_out=O[:, COL_D8 : COL_D8 + 1],
            )

            nc.sync.dma_start(out[:, :], O[:])

    nc.compile()
    return nc


def _get_nc():
    if "nc" not in _NC_CACHE:
        _NC_CACHE["nc"] = build_nc()
    return _NC_CACHE["nc"]


def make_in_maps(correlation_map_real, correlation_map_fake, fake_weight):
    r = np.ascontiguousarray(correlation_map_real, dtype=np.float32).reshape(B, FF)
    f = np.ascontiguousarray(correlation_map_fake, dtype=np.float32).reshape(
        B, PP, PP
    )
    w = np.ascontiguousarray(fake_weight, dtype=np.float32).reshape(B, PP)
    return [
        {
            "real": r[k * BS : (k + 1) * BS],
            "fake": f[k * BS : (k + 1) * BS],
            "fw": w[k * BS : (k + 1) * BS],
        }
        for k in range(NCORES)
    ]


def reduce_outputs(results):
    total = 0.0
    for k in range(NCORES):
        Ov = results[k]["out"].astype(np.float64)
        total += (
            Ov[:, COL_REAL : COL_REAL + RC].sum()
            + Ov[:, COL_FAKE : COL_FAKE + NOPS].sum()
            + Ov[:, COL_D1].sum()
            - Ov[:, COL_D8].sum()
        )
    return np.float32(total / DENOM)


def run(inputs, trace=False, **kwargs):
    nc = _get_nc()
    in_maps = make_in_maps(**inputs)
    res = bass_utils.run_bass_kernel_spmd(
        nc, in_maps, list(range(NCORES)), trace=trace, **kwargs
    )
    return reduce_outputs(res.results), res


def kernel(correlation_map_real, correlation_map_fake, fake_weight):
    loss, _ = run(
        dict(
            correlation_map_real=correlation_map_real,
            correlation_map_fake=correlation_map_fake,
            fake_weight=fake_weight,
        )
    )
    return loss


# revision 21
# speedup vs baseline: 1.2089x; 1.2089x over previous
"""Trainium2 Bass kernel for nn_Attention_Correlation_weight_reshape_loss.

loss = [ sum|real - C_real| + sum|fake - C_fake| ] / (B*(PP^2-PP))

Key identity: C_IN == C_OUT == 0.8, so with s[b,i] = +1 if fake_weight[b,i] > 0
else -1 the fake target is rank-1:
    C_fake[b,i,j] = 0.45 + 0.35 * s[b,i] * s[b,j]
C_real = 0.8 everywhere except the diagonal (1.0).

Per-core plan (data-parallel over batch, 8 cores x 128 batches):
  real map, layout [batch=partition, 38416 free], 14 chunks:
      ScalarE:  y = Abs(x - 0.8) with free-dim accum -> per-chunk column
      VectorE:  gather diagonal (stride 197) into a compact [128,196] tile
      final:    Abs-accum of |diag-1| and |diag-0.8| (host applies +/-)
  fake map, per-batch row-pair layout [98 partitions, 2 rows, 196] so HBM
  descriptors are 1568B (rows 2p,2p+1 are adjacent in memory):
      TensorE:  one K=2 bf16 matmul per batch: block-diagonal rhs emits
                sigma[p, h*196+j] = s[2p+h]*s[j] into one PSUM bank
      VectorE:  custom fused DVE op over 4 batches at a time:
                out = |sigma*(-0.35) - 0.45 + x|, accum_out = row sum
Host sums the [128, 48] partial tensor from each core and divides by denom.
"""

from operator import add as _op_add

import numpy as np

import concourse.bacc as bacc
import concourse.bass as bass
import concourse.mybir as mybir
import concourse.tile as tile
from concourse import bass_utils
from concourse import dve_ops as _dops
from concourse.dve_spec import Spec, Src0, Src1, Zero, maxx, lower
from concourse.dve_spec import _has_src1
from concourse import dve_spec as _dspec
from concourse.dve_uop import DveOpSpec
from concourse.dve_table_gen import dve_ver_for

F32 = mybir.dt.float32
BF16 = mybir.dt.bfloat16
AF = mybir.ActivationFunctionType
ALU = mybir.AluOpType

B, PP = 1024, 196
NCORES = 8
BS = B // NCORES            # 128 batches per core
FF = PP * PP                # 38416
RC = 14                     # real chunks
RF = FF // RC               # 2744
HALF = PP // 2              # 98
GRP = 8                     # fake batches per DMA group
NGRP = BS // GRP            # 16
DVB = 4                     # fake batches per custom-DVE op (PSUM banks)
NOPS = BS // DVB            # 32 fake accum columns

# output partials layout: [128, NCOL]
COL_REAL = 0                # RC cols: per-chunk sum|x-0.8| (incl diag)
COL_FAKE = RC               # 32 cols: per-4-batch fake sums (rows 0..97 only)
COL_D1 = RC + NOPS          # sum|diag-1.0|
COL_D8 = COL_D1 + 1        # sum|diag-0.8|
NCOL = COL_D8 + 1          # 48

DENOM = float(B) * (FF - PP)

_NC_CACHE = {}


def _register_abs_op():
    """Custom DVE op: out = |(in0*s0 - s1) + in1|, accum_out = row-sum(out)."""
    name = "ABS_AFFINE_SUM_ANT"
    for op in _dops.OPS:
        if op.name == name:
            return op
    e = (Src0 * _dspec.C0 - _dspec.C1) + Src1
    body = maxx(e, Zero - e)

    def _ref(in0, in1, c0, c1, c2):
        P = in0.shape[0]
        a = np.asarray(in0, dtype=np.float32).reshape(P, -1)
        x = np.asarray(in1, dtype=np.float32).reshape(P, -1)
        b = np.abs((a * c0 - c1) + x).astype(np.float32)
        return b, b.sum(axis=-1, keepdims=True)

    spec = Spec(body=body, accum=_op_add, accum_init=Zero, reference=_ref)
    row = max(_dops._SUB_OPCODE_FOR_NAME.values()) + 1
    assert row < 0x20
    _dops._SUB_OPCODE_FOR_NAME[name] = row
    shas = {}
    for ver in ("v3", "v4"):
        s = DveOpSpec(
            name=name, opcode=row, uops=lower(spec, ver=ver),
            rd1_en=_has_src1(spec),
        )
        shas[ver] = s.sha(ver)
    op = _dops.DveOp(name, spec, subdim=False, uops_sha=shas)
    _dops.OPS.append(op)
    _dops.CUSTOM_DVE_SPECS[name] = spec
    return op


def build_nc():
    abs_op = _register_abs_op()
    nc = bacc.Bacc(
        "TRN2", target_bir_lowering=False, debug=False, enable_asserts=False
    )
    real = nc.dram_tensor("real", [BS, FF], F32, kind="ExternalInput").ap()
    fake = nc.dram_tensor("fake", [BS, PP, PP], F32, kind="ExternalInput").ap()
    fw = nc.dram_tensor("fw", [BS, PP], F32, kind="ExternalInput").ap()
    out = nc.dram_tensor("out", [128, NCOL], F32, kind="ExternalOutput").ap()

    with tile.TileContext(nc) as tc:
        with (
            tc.tile_pool(name="small", bufs=1) as sp,
            tc.tile_pool(name="xr", bufs=3) as xr_pool,
            tc.tile_pool(name="xf", bufs=4) as xf_pool,
            tc.tile_pool(name="d", bufs=2) as d_pool,
            tc.tile_pool(name="ps", bufs=2, space="PSUM") as ps_pool,
        ):
            O = sp.tile([128, NCOL], F32)
            nc.gpsimd.memset(O[:], 0.0)

            # bias constants for scalar-engine activations ([P,1] APs)
            b08 = sp.tile([128, 1], F32)
            nc.gpsimd.memset(b08[:], -0.8)
            bm1 = sp.tile([128, 1], F32)
            nc.gpsimd.memset(bm1[:], -1.0)

            # --- s prep: s = +/-1 (bf16) from fw > 0.  All small DMAs go
            # through GPSIMD (SWDGE) so the sync/scalar HWDGE streams start
            # on the big loads immediately.
            fwt = sp.tile([128, PP], F32)
            nc.gpsimd.dma_start(fwt[:], fw[:, :])
            g_t = sp.tile([128, PP], F32)
            nc.vector.tensor_scalar(g_t[:], fwt[:], 0.0, None, ALU.is_gt)
            s_bf = sp.tile([128, PP], BF16)
            nc.vector.tensor_scalar(s_bf[:], g_t[:], 2.0, 1.0, ALU.mult, ALU.subtract)
            # even/odd row weights for the K=2 row-pair matmul, flattened
            # batch-major onto partitions 0 and 1
            e_t = sp.tile([128, HALF], BF16)
            nc.vector.tensor_copy(e_t[:], s_bf[:, 0 : PP - 1 : 2])
            o_t = sp.tile([128, HALF], BF16)
            nc.vector.tensor_copy(o_t[:], s_bf[:, 1:PP:2])
            lhsT2 = sp.tile([2, BS * HALF], BF16)
            nc.gpsimd.dma_start(lhsT2[0:1, :], e_t[:])
            nc.gpsimd.dma_start(lhsT2[1:2, :], o_t[:])

            # two persistent block-diagonal rhs tiles (zeros memset once;
            # per-group DMAs only overwrite the s blocks)
            rhs_tiles = []
            for _ in range(2):
                rt = sp.tile([2, GRP * 2 * PP], BF16, tag=f"rhs{_}")
                nc.gpsimd.memset(rt[:], 0.0)
                rhs_tiles.append(rt[:].rearrange("p (g j) -> p g j", g=GRP))

            diag = sp.tile([128, PP], F32)

            def real_chunk(c):
                xr = xr_pool.tile([128, RF], F32, tag="xr")
                nc.scalar.dma_start(xr[:], real[:, c * RF : (c + 1) * RF])
                # diagonal positions f = 197*i inside this chunk (extract
                # before the in-place Abs below overwrites xr)
                lo = c * RF
                i0 = -(-lo // 197)
                i1 = -(-(lo + RF) // 197)
                off = 197 * i0 - lo
                cnt = i1 - i0
                nc.vector.tensor_copy(
                    diag[:, i0:i1], xr[:, off : off + 197 * (cnt - 1) + 1 : 197]
                )
                nc.scalar.activation(
                    xr[:], xr[:], AF.Abs, bias=b08[:],
                    accum_out=O[:, COL_REAL + c : COL_REAL + c + 1],
                )

            def fake_dma(gr):
                b0 = gr * GRP
                # row-pair fold: xf[p, g, h, j] = fake[b0+g, 2p+h, j]
                xf = xf_pool.tile([HALF, GRP, 2, PP], F32, tag="xf")
                nc.sync.dma_start(
                    xf[:],
                    fake[b0 : b0 + GRP, :, :].rearrange(
                        "g (p h) j -> p g h j", h=2
                    ),
                )
                return xf

            def fake_compute(gr, xf):
                b0 = gr * GRP
                # block-diagonal rhs: per batch block [s_b | 0 ; 0 | s_b]
                r3 = rhs_tiles[gr % 2]
                nc.gpsimd.dma_start(r3[0:1, :, 0:PP], s_bf[b0 : b0 + GRP, :])
                nc.gpsimd.dma_start(
                    r3[1:2, :, PP : 2 * PP], s_bf[b0 : b0 + GRP, :]
                )
                for m in range(GRP // DVB):
                    ps4 = ps_pool.tile([HALF, DVB, 512], F32, tag="ps")
                    for bl in range(DVB):
                        gi = m * DVB + bl
                        b = b0 + gi
                        nc.tensor.matmul(
                            ps4[:, bl, 0 : 2 * PP],
                            lhsT2[0:2, b * HALF : (b + 1) * HALF],
                            r3[0:2, gi, :],
                            start=True, stop=True,
                        )
                    d = d_pool.tile([HALF, DVB, 2 * PP], F32, tag="d")
                    col = COL_FAKE + gr * (GRP // DVB) + m
                    nc.vector._custom_dve(
                        abs_op,
                        out=d[:],
                        in0=ps4[:, :, 0 : 2 * PP],
                        in1=xf[:, m * DVB : (m + 1) * DVB, :, :].rearrange(
                            "p g h j -> p g (h j)"
                        ),
                        s0=-0.35,
                        s1=0.45,
                        accum_out=O[0:HALF, col : col + 1],
                    )

            # interleave: 16 fake groups, RC real chunks spread 1-per-2-groups
            pending = []
            for i in range(NGRP):
                pending.append((i, fake_dma(i)))
                if i % 2 == 0 and i // 2 < RC:
                    real_chunk(i // 2)
                while pending:
                    gr, xf = pending.pop(0)
                    fake_compute(gr, xf)

            # diagonal corrections: real diag target is 1.0 (not 0.8)
            t1 = sp.tile([128, PP], F32)
            nc.scalar.activation(
                t1[:], diag[:], AF.Abs, bias=bm1[:],
                accum_out=O[:, COL_D1 : COL_D1 + 1],
            )
            t2 = sp.tile([128, PP], F32)
            nc.scalar.activation(
                t2[:], diag[:], AF.Abs, bias=b08[:],
                accum_out=O[:, COL_D8 : COL_D8 + 1],
            )

            nc.sync.dma_start(out[:, :], O[:])

    nc.compile()
    return nc


def _get_nc():
    if "nc" not in _NC_CACHE:
        _NC_CACHE["nc"] = build_nc()
    return _NC_CACHE["nc"]


def make_in_maps(correlation_map_real, correlation_map_fake, fake_weight):
    r = np.ascontiguousarray(correlation_map_real, dtype=np.float32).reshape(B, FF)
    f = np.ascontiguousarray(correlation_map_fake, dtype=np.float32).reshape(
        B, PP, PP
    )
    w = np.ascontiguousarray(fake_weight, dtype=np.float32).reshape(B, PP)
    return [
        {
            "real": r[k * BS : (k + 1) * BS],
            "fake": f[k * BS : (k + 1) * BS],
            "fw": w[k * BS : (k + 1) * BS],
        }
        for k in range(NCORES)
    ]


def reduce_outputs(results):
    total = 0.0
    for k in range(NCORES):
        Ov = results[k]["out"].astype(np.float64)
        total += (
            Ov[:, COL_REAL : COL_REAL + RC].sum()
            + Ov[:, COL_FAKE : COL_FAKE + NOPS].sum()
            + Ov[:, COL_D1].sum()
            - Ov[:, COL_D8].sum()
        )
    return np.float32(total / DENOM)


def run(inputs, trace=False, **kwargs):
    nc = _get_nc()
    in_maps = make_in_maps(**inputs)
    res = bass_utils.run_bass_kernel_spmd(
        nc, in_maps, list(range(NCORES)), trace=trace, **kwargs
    )
    return reduce_outputs(res.results), res


def kernel(correlation_map_real, correlation_map_fake, fake_weight):
    loss, _ = run(
        dict(
            correlation_map_real=correlation_map_real,
            correlation_map_fake=correlation_map_fake,
            fake_weight=fake_weight,
        )
    )
    return loss
